# revision 1
# baseline (speedup 1.0000x reference)
"""AFT attention kernel v2 for Trainium2, data-parallel over batch on 8 cores.

Math per batch element b:
    proj = x @ w_attn ; q, k, v = split(proj)
    ke = exp(k - rowmax k); kv = ke * v
    EB[i,j] = exp(pos_bias[i,j]) * (j <= i)
    num = EB @ kv ; den = EB @ ke
    y = sigmoid(q) * num / den ; out = y @ w_proj

v2 restructure of the EB matmuls (the O(T^2 D) part). Since
pos_bias ~ N(0, 0.02^2), EB = tril(1) + tril(exp(pb) - 1) where the second
term is ~2% of the signal:
  - tril(1) part: per-128-block running prefix.  For i-block g:
    num_main = carry_g + Ltri128^T @ kv[block g] where carry_g = sum of kv
    over blocks < g, accumulated in a dedicated PSUM bank by all-ones
    stationary matmuls (one per block), snapshotted to SBUF between blocks.
  - correction: EBm1 = exp(pb)-1 masked, quantized to fp8e4 (error ~4% of a
    ~2% term -> ~0.1% of output), run with DoubleRow perf mode at 0.5
    cycles/row, contracting two 128-j-blocks per instruction.

Everything runs in [i/t-partition, d-free] orientation so the carry is a
per-partition broadcast add fused into the eviction.  y must be transposed
(PE transpose-matmuls) for the output projection, which keeps the baseline's
[d-part, t] proj structure.

Phase C runs twice (d-halves) so block+carry PSUM fits in 8 banks; phase B
(sigmoid(q), also flipped to [t, d]) runs per-half right before each pass so
only half of sq is ever resident.
"""

import numpy as np

import concourse.mybir as mybir
import concourse.tile as tile
from concourse import bacc
from concourse.bass import ts, ds
from concourse.bass_utils import run_bass_kernel_spmd

F32 = mybir.dt.float32
BF16 = mybir.dt.bfloat16
FP8 = mybir.dt.float8e4
X = mybir.AxisListType.X
MUL = mybir.AluOpType.mult
ADD = mybir.AluOpType.add
MIN = mybir.AluOpType.min
DR = mybir.MatmulPerfMode.DoubleRow

P = 128
B, T, D = 8, 2048, 1024
NDC = D // P  # 8 d-chunks of 128 (contraction)
NTB = T // P  # 16 t/i-blocks of 128
H = 512  # d-half size


def build_nc(reps=1):
    nc = bacc.Bacc("TRN2")

    xTb = nc.declare_dram_parameter("xTb", [D, T], BF16, isOutput=False)
    wq = nc.declare_dram_parameter("wq", [D, D], BF16, isOutput=False)
    wk = nc.declare_dram_parameter("wk", [D, D], BF16, isOutput=False)
    wv = nc.declare_dram_parameter("wv", [D, D], BF16, isOutput=False)
    wp = nc.declare_dram_parameter("wp", [D, D], BF16, isOutput=False)
    pb8 = nc.declare_dram_parameter("pb8", [T, T], FP8, isOutput=False)
    # consts[:, 0, :] = ltri (ltri[j,i] = j<=i), [:, 1, :] = identity,
    # [:, 2, :] = ones
    consts = nc.declare_dram_parameter("consts", [P, 3, P], BF16, isOutput=False)
    out = nc.declare_dram_parameter("out", [T, D], F32, isOutput=True)

    with tile.TileContext(nc) as tc:
        _emit(nc, tc, xTb, wq, wk, wv, wp, pb8, consts, out, reps=reps)
    nc.compile()
    return nc


def _emit(nc, tc, xTb, wq, wk, wv, wp, pb8, consts, out, reps=1):
    import contextlib

    ctx = contextlib.ExitStack()
    with ctx:
        singles = ctx.enter_context(tc.tile_pool(name="singles", bufs=1))
        u16 = ctx.enter_context(tc.tile_pool(name="u16", bufs=6))
        eb8p = ctx.enter_context(tc.tile_pool(name="eb8p", bufs=3))
        carp = ctx.enter_context(tc.tile_pool(name="carp", bufs=2))
        drp = ctx.enter_context(tc.tile_pool(name="drp", bufs=2))
        ytp = ctx.enter_context(tc.tile_pool(name="ytp", bufs=3))
        yt1p = ctx.enter_context(tc.tile_pool(name="yt1p", bufs=2))
        outp = ctx.enter_context(tc.tile_pool(name="outp", bufs=2))
        smallp = ctx.enter_context(tc.tile_pool(name="smallp", bufs=4))
        pmain = ctx.enter_context(tc.tile_pool(name="pmain", bufs=6, space="PSUM"))
        pop = ctx.enter_context(tc.tile_pool(name="pop", bufs=2, space="PSUM"))

        xTb_r = xTb[:].rearrange("(dc p) t -> p dc t", p=P)

        # long-lived SBUF
        kvke = singles.tile([P, NTB, 2 * D], BF16, tag="kvke")
        kvke8 = singles.tile([P, NTB, 2 * D], FP8, tag="kvke8")
        sqh = singles.tile([P, NTB, H], BF16, tag="sqh")  # sigmoid(q) d-half
        yT0 = singles.tile([P, 4, T], BF16, tag="yT0")  # yT d-chunks 0..3
        csts = singles.tile([P, 3, P], BF16, tag="csts")
        nc.sync.dma_start(out=csts[:], in_=consts[:])
        ltri = csts[:, 0, :]
        ident = csts[:, 1, :]
        onesb = csts[:, 2, :]

        _wn = [0]

        def wtile():
            _wn[0] += 1
            return u16.tile([P, NDC, H], BF16, tag="u", name=f"w{_wn[0]}")

        def wload(dst, src_handle, half):
            nc.sync.dma_start(
                out=dst[:],
                in_=src_handle[:].rearrange("(dc p) f -> p dc f", p=P)[
                    :, :, ts(half, H)
                ],
            )

        for _rep in range(reps):
            # ---------------- Phase A: kvke = [ke*v | ke], + fp8 copy --------
            wk_h = [wtile() for _ in range(2)]
            wv_h = [wtile() for _ in range(2)]
            wload(wk_h[0], wk, 0)

            for tblk in range(4):
                xt_t = u16.tile([P, NDC, H], BF16, tag="u")
                if tblk == 0:
                    nc.sync.dma_start(out=xt_t[:, :, :128], in_=xTb_r[:, :, :128])
                    nc.sync.dma_start(out=xt_t[:, :, 128:256], in_=xTb_r[:, :, 128:256])
                    wload(wk_h[1], wk, 1)
                    nc.sync.dma_start(out=xt_t[:, :, 256:], in_=xTb_r[:, :, 256:512])
                    wload(wv_h[0], wv, 0)
                    wload(wv_h[1], wv, 1)
                else:
                    nc.sync.dma_start(out=xt_t[:], in_=xTb_r[:, :, ts(tblk, H)])
                for sub in range(4):
                    tb = tblk * 4 + sub
                    if tblk in (0, 3):
                        # tblk 0: all k before any v (wv still streaming);
                        # tblk 3: k first so wk slots free for prefetch.
                        order = [("k", 0), ("k", 1), ("v", 0), ("v", 1)]
                    else:
                        order = [("k", 0), ("v", 0), ("k", 1), ("v", 1)]
                    pk, pv = {}, {}
                    for kind, half in order:
                        ps = pmain.tile([P, H], F32, tag="ps")
                        w = (wk_h if kind == "k" else wv_h)[half]
                        for dc in range(NDC):
                            nc.tensor.matmul(
                                ps[:],
                                xt_t[:, dc, ts(sub, P)],
                                w[:, dc, :],
                                start=(dc == 0),
                                stop=(dc == NDC - 1),
                            )
                        (pk if kind == "k" else pv)[half] = ps
                    m0 = smallp.tile([P, 1], F32, tag="m0")
                    m1 = smallp.tile([P, 1], F32, tag="m1")
                    nc.vector.reduce_max(m0[:], pk[0][:], axis=X, negate=True)
                    nc.vector.reduce_max(m1[:], pk[1][:], axis=X, negate=True)
                    nm = smallp.tile([P, 1], F32, tag="nm")
                    nc.vector.tensor_tensor(nm[:], m0[:], m1[:], op=MIN)
                    for half in range(2):
                        nc.scalar.activation(
                            out=kvke[:, tb, ds(D + half * H, H)],
                            in_=pk[half][:],
                            func=mybir.ActivationFunctionType.Exp,
                            bias=nm[:],
                        )
                        nc.vector.tensor_tensor(
                            kvke[:, tb, ds(half * H, H)],
                            pv[half][:],
                            kvke[:, tb, ds(D + half * H, H)],
                            op=MUL,
                        )
                        nc.gpsimd.tensor_copy(
                            kvke8[:, tb, ds(D + half * H, H)],
                            kvke[:, tb, ds(D + half * H, H)],
                        )
                        nc.gpsimd.tensor_copy(
                            kvke8[:, tb, ds(half * H, H)],
                            kvke[:, tb, ds(half * H, H)],
                        )

            # ---------------- Phases B+C per d-half ------------------------
            wp_h = None
            for ph in range(2):
                d0 = ph * H

                # Phase B (flipped): sq[t, d0:d0+512] for all 16 t-blocks
                wq_h = wtile()
                wload(wq_h, wq, ph)
                for tblk in range(4):
                    xt_t = u16.tile([P, NDC, H], BF16, tag="u")
                    nc.sync.dma_start(out=xt_t[:], in_=xTb_r[:, :, ts(tblk, H)])
                    for gi in range(4):
                        g = tblk * 4 + gi
                        ps = pmain.tile([P, H], F32, tag="ps")
                        for dc in range(NDC):
                            nc.tensor.matmul(
                                ps[:],
                                xt_t[:, dc, ts(gi, P)],
                                wq_h[:, dc, :],
                                start=(dc == 0),
                                stop=(dc == NDC - 1),
                            )
                        nc.scalar.activation(
                            out=sqh[:, g, :],
                            in_=ps[:],
                            func=mybir.ActivationFunctionType.Sigmoid,
                        )

                if ph == 1 and wp_h is None:
                    wp_h = [wtile() for _ in range(2)]
                    wload(wp_h[0], wp, 0)
                    wload(wp_h[1], wp, 1)

                # Phase C pass over i-blocks
                def eb_load(g):
                    nj2 = 2 * (g // 2 + 1)
                    t8 = eb8p.tile([P, NTB, P], FP8, tag="eb8")
                    nc.sync.dma_start(
                        out=t8[:, :nj2, :],
                        in_=pb8[0 : nj2 * P, ts(g, P)].rearrange(
                            "(b p) i -> p b i", p=P
                        ),
                    )
                    return t8

                eb_t = eb_load(0)
                csb_prev = None
                pend = None  # deferred (transpose+evict+proj) for g-1
                for g in range(NTB):
                    npairs = g // 2 + 1
                    bn = pmain.tile([P, H], F32, tag="ps")
                    bd = pmain.tile([P, H], F32, tag="ps")
                    for jp in range(npairs):
                        s = 2 * jp
                        nc.tensor.matmul(
                            bn[:],
                            eb_t[:, s : s + 2, :],
                            kvke8[:, s : s + 2, ds(d0, H)],
                            perf_mode=DR,
                            start=(jp == 0),
                            stop=False,
                        )
                        nc.tensor.matmul(
                            bd[:],
                            eb_t[:, s : s + 2, :],
                            kvke8[:, s : s + 2, ds(D + d0, H)],
                            perf_mode=DR,
                            start=(jp == 0),
                            stop=False,
                        )
                    nc.tensor.matmul(
                        bn[:], ltri, kvke[:, g, ds(d0, H)], start=False, stop=True
                    )
                    nc.tensor.matmul(
                        bd[:], ltri, kvke[:, g, ds(D + d0, H)], start=False, stop=True
                    )

                    if g + 1 < NTB:
                        eb_t = eb_load(g + 1)

                    # block sums S(g) (closed psum groups); the running carry
                    # csb_g = sum over blocks <= g lives in SBUF f32 via a
                    # vector chain.  evict(g) uses csb_{g-1}.
                    csb = csb_prev
                    if g < NTB - 1:
                        s_kv = pmain.tile([P, H], F32, tag="ps")
                        s_ke = pmain.tile([P, H], F32, tag="ps")
                        nc.tensor.matmul(s_kv[:], onesb, kvke[:, g, ds(d0, H)])
                        nc.tensor.matmul(s_ke[:], onesb, kvke[:, g, ds(D + d0, H)])
                        csb_new = carp.tile([P, 2, H], F32, tag="csb")
                        if csb is None:
                            nc.vector.tensor_copy(csb_new[:, 0, :], s_kv[:])
                            nc.vector.tensor_copy(csb_new[:, 1, :], s_ke[:])
                        else:
                            nc.vector.tensor_tensor(
                                csb_new[:, 0, :], csb[:, 0, :], s_kv[:], op=ADD
                            )
                            nc.vector.tensor_tensor(
                                csb_new[:, 1, :], csb[:, 1, :], s_ke[:], op=ADD
                            )
                        csb_prev = csb_new

                    # flush deferred PE work for g-1 (issued after g's main
                    # matmuls so the in-order PE queue never waits on the
                    # vector eviction of g-1)
                    if pend is not None:
                        _transp_proj(
                            nc, tc, pend, yT0, yt1p, pop, outp,
                            wp_h, ident, out, ph,
                        )
                        pend = None

                    # eviction: y[t-block g, d-half] = sig(q)*(num/den)
                    dr = drp.tile([P, H], F32, tag="dr")
                    if csb is not None:
                        nc.vector.tensor_tensor(bd[:], bd[:], csb[:, 1, :], op=ADD)
                        nc.vector.tensor_tensor(bn[:], bn[:], csb[:, 0, :], op=ADD)
                    nc.vector.reciprocal(dr[:], bd[:])
                    yt = ytp.tile([P, H], BF16, tag="yt")
                    nc.vector.tensor_tensor(yt[:], bn[:], dr[:], op=MUL)
                    nc.vector.tensor_tensor(yt[:], yt[:], sqh[:, g, :], op=MUL)
                    pend = (g, yt)

                _transp_proj(
                    nc, tc, pend, yT0, yt1p, pop, outp, wp_h, ident, out, ph
                )


def _transp_proj(nc, tc, pend, yT0, yt1p, pop, outp, wp_h, ident, out, ph):
    g, yt = pend
    pyt = pop.tile([P, 4, P], BF16, tag="po")
    for c in range(4):
        nc.tensor.matmul(
            pyt[:, c, :], yt[:, ts(c, P)], ident, is_transpose=True
        )
    if ph == 0:
        nc.scalar.copy(out=yT0[:, :, ts(g, P)], in_=pyt[:])
        return
    yt1 = yt1p.tile([P, 4, P], BF16, tag="yt1")
    nc.scalar.copy(out=yt1[:], in_=pyt[:])
    for oh in range(2):
        po = pop.tile([P, H], F32, tag="po")
        for dc in range(NDC):
            lhsT = yT0[:, dc, ts(g, P)] if dc < 4 else yt1[:, dc - 4, :]
            nc.tensor.matmul(
                po[:],
                lhsT,
                wp_h[oh][:, dc, :],
                start=(dc == 0),
                stop=(dc == NDC - 1),
            )
        o_t = outp.tile([P, H], F32, tag="o")
        if oh == 0:
            nc.scalar.copy(out=o_t[:], in_=po[:])
        else:
            nc.vector.tensor_copy(o_t[:], po[:])
        nc.sync.dma_start(out=out[ts(g, P), ts(oh, H)], in_=o_t[:])


def make_in_maps(x, w_attn, w_proj, pos_bias):
    import ml_dtypes

    bf = ml_dtypes.bfloat16
    f8 = ml_dtypes.float8_e4m3
    xT_all = np.ascontiguousarray(np.transpose(np.asarray(x, np.float32), (0, 2, 1)))
    xTb_all = xT_all.astype(bf)
    w_attn = np.asarray(w_attn, np.float32)
    wq = np.ascontiguousarray(w_attn[:, :D]).astype(bf)
    wk = np.ascontiguousarray(w_attn[:, D : 2 * D]).astype(bf)
    wv = np.ascontiguousarray(w_attn[:, 2 * D :]).astype(bf)
    wp = np.ascontiguousarray(np.asarray(w_proj, np.float32)).astype(bf)

    # EBm1 = exp(pos_bias) - 1, transposed to [j, i], upper (j > i) zeroed.
    pb = np.asarray(pos_bias, np.float32)
    ebm1 = (np.exp(pb) - 1.0).T.copy()
    jj = np.arange(T)[:, None]
    ii = np.arange(T)[None, :]
    ebm1[jj > ii] = 0.0
    pb8 = ebm1.astype(f8)

    consts = np.zeros((P, 3, P), np.float32)
    consts[:, 0, :] = (np.arange(P)[:, None] <= np.arange(P)[None, :])  # ltri
    consts[:, 1, :] = np.eye(P)
    consts[:, 2, :] = 1.0
    consts = consts.astype(bf)

    shared = dict(wq=wq, wk=wk, wv=wv, wp=wp, pb8=pb8, consts=consts)
    return [dict(xTb=xTb_all[i], **shared) for i in range(B)]


_NC_CACHE = {}


def get_nc():
    if "nc" not in _NC_CACHE:
        _NC_CACHE["nc"] = build_nc()
    return _NC_CACHE["nc"]


def kernel(x, w_attn, w_proj, pos_bias):
    nc = get_nc()
    in_maps = make_in_maps(x, w_attn, w_proj, pos_bias)
    res = run_bass_kernel_spmd(nc, in_maps, core_ids=list(range(B)))
    return np.stack([res.results[i]["out"] for i in range(B)]).astype(np.float32)



# revision 2
# speedup vs baseline: 1.1786x; 1.1786x over previous
"""AFT attention kernel v4 for Trainium2, data-parallel over batch on 8 cores.

Math per batch element b:
    proj = x @ w_attn ; q, k, v = split(proj)
    ke = exp(k - rowmax k); kv = ke * v
    EB[i,j] = exp(pos_bias[i,j]) * (j <= i)
    num = EB @ kv ; den = EB @ ke
    y = sigmoid(q) * num / den ; out = y @ w_proj

v4 = v3 + phase-B interleaving.  The TRN2 PE p-state throttles to 1.2 GHz
after any idle gap and needs ~3us of continuous execution to return to
2.4 GHz; measured HW time for the sparse phases matched the mid clock almost
exactly, and a filler-matmul probe (v3f) ran FASTER with extra always-ready
PE work.  So v4 removes the standalone sigmoid(q) pass and interleaves its
16x2 projection groups into phase C ph0 as always-ready PE work that bridges
the dependency edges between blocks: per i-block the PE stream is
[DR corr + ltri + carry matmuls (~2us)] [2 q-projection groups (~3.4us)],
which keeps the engine dense end to end.  Carry accumulators for kv and ke
are packed into one PSUM bank (partitions 0 and 32), freeing a bank so
bn/bd/q-proj groups rotate through a 5-buf pool.

From v3: PSUM prefix carry via M=1 ones-matmuls + Act [1,512] snapshot +
K=1 broadcast-add matmuls (no DVE carry chains); x streamed in 8
quarter-tiles; bf16 output; fp8 DoubleRow correction for exp(pos_bias)-1.
"""

import numpy as np

import concourse.mybir as mybir
import concourse.tile as tile
from concourse import bacc
from concourse.bass import ts, ds
from concourse.bass_utils import run_bass_kernel_spmd

F32 = mybir.dt.float32
BF16 = mybir.dt.bfloat16
FP8 = mybir.dt.float8e4
X = mybir.AxisListType.X
MUL = mybir.AluOpType.mult
ADD = mybir.AluOpType.add
MIN = mybir.AluOpType.min
DR = mybir.MatmulPerfMode.DoubleRow

P = 128
B, T, D = 8, 2048, 1024
NDC = D // P  # 8 d-chunks of 128 (contraction)
NTB = T // P  # 16 t/i-blocks of 128
NTQ = 8  # t quarters of 256
H = 512  # d-half size


def build_nc(reps=1):
    nc = bacc.Bacc("TRN2")

    xTb = nc.declare_dram_parameter("xTb", [D, T], BF16, isOutput=False)
    wq = nc.declare_dram_parameter("wq", [D, D], BF16, isOutput=False)
    wk = nc.declare_dram_parameter("wk", [D, D], BF16, isOutput=False)
    wv = nc.declare_dram_parameter("wv", [D, D], BF16, isOutput=False)
    wp = nc.declare_dram_parameter("wp", [D, D], BF16, isOutput=False)
    pb8 = nc.declare_dram_parameter("pb8", [T, T], FP8, isOutput=False)
    # consts[:, 0, :] = ltri (ltri[j,i] = j<=i), [:, 1, :] = identity,
    # [:, 2, :] = ones
    consts = nc.declare_dram_parameter("consts", [P, 3, P], BF16, isOutput=False)
    out = nc.declare_dram_parameter("out", [T, D], BF16, isOutput=True)

    with tile.TileContext(nc) as tc:
        _emit(nc, tc, xTb, wq, wk, wv, wp, pb8, consts, out, reps=reps)
    nc.compile()
    return nc


def _emit(nc, tc, xTb, wq, wk, wv, wp, pb8, consts, out, reps=1):
    import contextlib

    ctx = contextlib.ExitStack()
    with ctx:
        singles = ctx.enter_context(tc.tile_pool(name="singles", bufs=1))
        wpool = ctx.enter_context(tc.tile_pool(name="wpool", bufs=4))
        xpool = ctx.enter_context(tc.tile_pool(name="xpool", bufs=3))
        eb8p = ctx.enter_context(tc.tile_pool(name="eb8p", bufs=3))
        csbp = ctx.enter_context(tc.tile_pool(name="csbp", bufs=1))
        drp = ctx.enter_context(tc.tile_pool(name="drp", bufs=1))
        ytp = ctx.enter_context(tc.tile_pool(name="ytp", bufs=2))
        yt1p = ctx.enter_context(tc.tile_pool(name="yt1p", bufs=2))
        outp = ctx.enter_context(tc.tile_pool(name="outp", bufs=2))
        smallp = ctx.enter_context(tc.tile_pool(name="smallp", bufs=4))
        pmain = ctx.enter_context(tc.tile_pool(name="pmain", bufs=5, space="PSUM"))
        pop = ctx.enter_context(tc.tile_pool(name="pop", bufs=2, space="PSUM"))
        pcar = ctx.enter_context(tc.tile_pool(name="pcar", bufs=1, space="PSUM"))

        xTb_r = xTb[:].rearrange("(dc p) t -> p dc t", p=P)

        # long-lived SBUF
        kvke = singles.tile([P, NTB, 2 * D], BF16, tag="kvke")
        kvke8 = singles.tile([P, NTB, 2 * D], FP8, tag="kvke8")
        sq = singles.tile([P, NTB, D], BF16, tag="sq")  # sigmoid(q), full d
        yT0 = singles.tile([P, 4, T], BF16, tag="yT0")  # yT d-chunks 0..3
        csts = singles.tile([P, 3, P], BF16, tag="csts")
        nc.sync.dma_start(out=csts[:], in_=consts[:])
        ltri = csts[:, 0, :]
        ident = csts[:, 1, :]
        ones_m1 = csts[:, 2, 0:1]  # [K=128, M=1] for carry accumulation
        ones_k1 = csts[0:1, 2, :]  # [K=1, M=128] for carry broadcast-add

        _wn = [0]

        def wtile():
            _wn[0] += 1
            return wpool.tile([P, NDC, H], BF16, tag="w", name=f"w{_wn[0]}")

        def wload(dst, src_handle, half):
            nc.sync.dma_start(
                out=dst[:],
                in_=src_handle[:].rearrange("(dc p) f -> p dc f", p=P)[
                    :, :, ts(half, H)
                ],
            )

        def eb_load(g):
            nj2 = 2 * (g // 2 + 1)
            t8 = eb8p.tile([P, NTB, P], FP8, tag="eb8")
            nc.sync.dma_start(
                out=t8[:, :nj2, :],
                in_=pb8[0 : nj2 * P, ts(g, P)].rearrange("(b p) i -> p b i", p=P),
            )
            return t8

        # state threaded across the rep loop phases
        for _rep in range(reps):
            # ---------------- Phase A: kvke = [ke*v | ke], + fp8 copy --------
            wk_h = [wtile() for _ in range(2)]
            wv_h = [wtile() for _ in range(2)]
            wload(wk_h[0], wk, 0)

            for tq in range(NTQ):
                xt_t = xpool.tile([P, NDC, 256], BF16, tag="x")
                if tq == 0:
                    nc.sync.dma_start(out=xt_t[:, :, :128], in_=xTb_r[:, :, :128])
                    wload(wk_h[1], wk, 1)
                    nc.sync.dma_start(out=xt_t[:, :, 128:], in_=xTb_r[:, :, 128:256])
                    wload(wv_h[0], wv, 0)
                    wload(wv_h[1], wv, 1)
                else:
                    nc.sync.dma_start(out=xt_t[:], in_=xTb_r[:, :, ts(tq, 256)])
                for sub in range(2):
                    tb = tq * 2 + sub
                    if tq in (0, 6, 7):
                        # tq 0: all k before any v (wv still streaming);
                        # tq 6/7: k first so wk slots free early for the wq
                        # prefetch at phase C.
                        order = [("k", 0), ("k", 1), ("v", 0), ("v", 1)]
                    else:
                        order = [("k", 0), ("v", 0), ("k", 1), ("v", 1)]
                    pk, pv = {}, {}
                    for kind, half in order:
                        ps = pmain.tile([P, H], F32, tag="ps")
                        w = (wk_h if kind == "k" else wv_h)[half]
                        for dc in range(NDC):
                            nc.tensor.matmul(
                                ps[:],
                                xt_t[:, dc, ts(sub, P)],
                                w[:, dc, :],
                                start=(dc == 0),
                                stop=(dc == NDC - 1),
                            )
                        (pk if kind == "k" else pv)[half] = ps
                    m0 = smallp.tile([P, 1], F32, tag="m0")
                    m1 = smallp.tile([P, 1], F32, tag="m1")
                    nc.vector.reduce_max(m0[:], pk[0][:], axis=X, negate=True)
                    nc.vector.reduce_max(m1[:], pk[1][:], axis=X, negate=True)
                    nm = smallp.tile([P, 1], F32, tag="nm")
                    nc.vector.tensor_tensor(nm[:], m0[:], m1[:], op=MIN)
                    for half in range(2):
                        nc.scalar.activation(
                            out=kvke[:, tb, ds(D + half * H, H)],
                            in_=pk[half][:],
                            func=mybir.ActivationFunctionType.Exp,
                            bias=nm[:],
                        )
                        nc.vector.tensor_tensor(
                            kvke[:, tb, ds(half * H, H)],
                            pv[half][:],
                            kvke[:, tb, ds(D + half * H, H)],
                            op=MUL,
                        )
                        nc.gpsimd.tensor_copy(
                            kvke8[:, tb, ds(D + half * H, H)],
                            kvke[:, tb, ds(D + half * H, H)],
                        )
                        nc.gpsimd.tensor_copy(
                            kvke8[:, tb, ds(half * H, H)],
                            kvke[:, tb, ds(half * H, H)],
                        )

            # ------- phase C prep: wq/wp prefetch, eb(0), q-proj state ------
            wq_h = [wtile() for _ in range(2)]
            wload(wq_h[0], wq, 0)
            wload(wq_h[1], wq, 1)
            wp_h = [wtile() for _ in range(2)]
            wload(wp_h[0], wp, 0)
            wload(wp_h[1], wp, 1)

            # interleaved q-projection groups. The half-0 stream runs 2
            # blocks ahead of the half-1 stream (1 quarter apart), so a
            # shared 2-deep quarter cache serves both with one DMA each.
            qx = {}

            def b_group(g, half):
                tq = g // 2
                if tq not in qx:
                    xt = xpool.tile([P, NDC, 256], BF16, tag="x", name="xt_b")
                    nc.sync.dma_start(out=xt[:], in_=xTb_r[:, :, ts(tq, 256)])
                    qx[tq] = xt
                    while len(qx) > 2:  # keep the 2 most recently inserted
                        del qx[next(iter(qx))]
                xt = qx[tq]
                ps = pmain.tile([P, H], F32, tag="ps")
                for dc in range(NDC):
                    nc.tensor.matmul(
                        ps[:],
                        xt[:, dc, ts(g % 2, P)],
                        wq_h[half][:, dc, :],
                        start=(dc == 0),
                        stop=(dc == NDC - 1),
                    )
                nc.scalar.activation(
                    out=sq[:, g, ds(half * H, H)],
                    in_=ps[:],
                    func=mybir.ActivationFunctionType.Sigmoid,
                )

            # depth-2 eb prefetch queue across both halves of this rep
            eb_queue = []
            eb_idx = [0]

            def eb_prefetch():
                i = eb_idx[0]
                if i < 2 * NTB:
                    eb_queue.append(eb_load(i % NTB))
                    eb_idx[0] = i + 1

            eb_prefetch()
            eb_prefetch()
            # prologue: 2 q-proj groups so sq[0..1, h0] are ready for the
            # first evictions, and the A->C boundary stays PE-dense
            b_group(0, 0)
            b_group(1, 0)

            # ---------------- Phase C per d-half ----------------------------
            for ph in range(2):
                d0 = ph * H

                # per-half PSUM carry accumulators, packed into one bank:
                # partition 0 = kv carry, partition 32 = ke carry (PE tiling
                # needs the out base partition in {0, 32, 64, 96})
                car = pcar.tile([P, H], F32, tag="car", name="car")
                ckv = car[0:1, :]
                cke = car[32:33, :]

                pend = None  # deferred (transpose+evict+proj) for g-1
                for g in range(NTB):
                    eb_t = eb_queue.pop(0)
                    npairs = g // 2 + 1

                    # snapshot the carry (state after block g-1) to SBUF so
                    # the PE can broadcast-add it into this block's groups;
                    # issued early so it lands during the DR matmuls below.
                    if g > 0:
                        csb = csbp.tile([P, 2, H], BF16, tag="csb")
                        nc.scalar.copy(out=csb[0:1, 0, :], in_=ckv[:])
                        nc.scalar.copy(out=csb[0:1, 1, :], in_=cke[:])

                    bn = pmain.tile([P, H], F32, tag="ps")
                    bd = pmain.tile([P, H], F32, tag="ps")
                    for jp in range(npairs):
                        s = 2 * jp
                        nc.tensor.matmul(
                            bn[:],
                            eb_t[:, s : s + 2, :],
                            kvke8[:, s : s + 2, ds(d0, H)],
                            perf_mode=DR,
                            start=(jp == 0),
                            stop=False,
                        )
                        nc.tensor.matmul(
                            bd[:],
                            eb_t[:, s : s + 2, :],
                            kvke8[:, s : s + 2, ds(D + d0, H)],
                            perf_mode=DR,
                            start=(jp == 0),
                            stop=False,
                        )
                    nc.tensor.matmul(
                        bn[:], ltri, kvke[:, g, ds(d0, H)],
                        start=False, stop=(g == 0),
                    )
                    nc.tensor.matmul(
                        bd[:], ltri, kvke[:, g, ds(D + d0, H)],
                        start=False, stop=(g == 0),
                    )
                    if g > 0:
                        nc.tensor.matmul(
                            bn[:], ones_k1, csb[0:1, 0, :],
                            start=False, stop=True,
                        )
                        nc.tensor.matmul(
                            bd[:], ones_k1, csb[0:1, 1, :],
                            start=False, stop=True,
                        )

                    # fold this block into the carry accumulators.  stop=True
                    # each time (a sim-only flag, no-op on HW) so the Act
                    # snapshot above never reads an open accumulation group;
                    # start=False still accumulates, so skip the group check.
                    if g < NTB - 1:
                        nc.tensor.matmul(
                            ckv, ones_m1, kvke[:, g, ds(d0, H)],
                            start=(g == 0), stop=True,
                            skip_group_check=(g > 0),
                        )
                        nc.tensor.matmul(
                            cke, ones_m1, kvke[:, g, ds(D + d0, H)],
                            start=(g == 0), stop=True,
                            skip_group_check=(g > 0),
                        )

                    # keep the eb prefetch queue 2 deep (crosses halves)
                    eb_prefetch()

                    # interleaved q-projection groups (always-ready PE work
                    # bridging dependency edges). ph0 carries the half-0
                    # stream (+2 lead) and the first half of the half-1
                    # stream; ph1 takes the rest, bridging its proj chains.
                    if ph == 0:
                        if g < NTB - 2:
                            b_group(g + 2, 0)
                        if g >= 8:
                            b_group(g - 8, 1)
                    else:
                        if g < 8:
                            b_group(g + 8, 1)

                    # flush deferred PE work for g-1 (issued after g's main
                    # matmuls so the in-order PE queue never waits on the
                    # vector eviction of g-1)
                    if pend is not None:
                        _transp_proj(
                            nc, tc, pend, yT0, yt1p, pop, outp,
                            wp_h, ident, out, ph,
                        )
                        pend = None

                    # eviction: y[t-block g, d-half] = sig(q)*(num/den)
                    dr = drp.tile([P, H], F32, tag="dr")
                    nc.vector.reciprocal(dr[:], bd[:])
                    yt = ytp.tile([P, H], BF16, tag="yt")
                    nc.vector.tensor_tensor(yt[:], bn[:], dr[:], op=MUL)
                    nc.vector.tensor_tensor(
                        yt[:], yt[:], sq[:, g, ds(d0, H)], op=MUL
                    )
                    pend = (g, yt)

                _transp_proj(
                    nc, tc, pend, yT0, yt1p, pop, outp, wp_h, ident, out, ph
                )


def _transp_proj(nc, tc, pend, yT0, yt1p, pop, outp, wp_h, ident, out, ph):
    g, yt = pend
    pyt = pop.tile([P, 4, P], BF16, tag="po")
    for c in range(4):
        nc.tensor.matmul(
            pyt[:, c, :], yt[:, ts(c, P)], ident, is_transpose=True
        )
    if ph == 0:
        nc.scalar.copy(out=yT0[:, :, ts(g, P)], in_=pyt[:])
        return
    yt1 = yt1p.tile([P, 4, P], BF16, tag="yt1")
    nc.scalar.copy(out=yt1[:], in_=pyt[:])
    for oh in range(2):
        po = pop.tile([P, H], F32, tag="po")
        for dc in range(NDC):
            lhsT = yT0[:, dc, ts(g, P)] if dc < 4 else yt1[:, dc - 4, :]
            nc.tensor.matmul(
                po[:],
                lhsT,
                wp_h[oh][:, dc, :],
                start=(dc == 0),
                stop=(dc == NDC - 1),
            )
        o_t = outp.tile([P, H], BF16, tag="o")
        if oh == 0:
            nc.scalar.copy(out=o_t[:], in_=po[:])
        else:
            nc.vector.tensor_copy(o_t[:], po[:])
        nc.sync.dma_start(out=out[ts(g, P), ts(oh, H)], in_=o_t[:])


def make_in_maps(x, w_attn, w_proj, pos_bias):
    import ml_dtypes

    bf = ml_dtypes.bfloat16
    f8 = ml_dtypes.float8_e4m3
    xT_all = np.ascontiguousarray(np.transpose(np.asarray(x, np.float32), (0, 2, 1)))
    xTb_all = xT_all.astype(bf)
    w_attn = np.asarray(w_attn, np.float32)
    wq = np.ascontiguousarray(w_attn[:, :D]).astype(bf)
    wk = np.ascontiguousarray(w_attn[:, D : 2 * D]).astype(bf)
    wv = np.ascontiguousarray(w_attn[:, 2 * D :]).astype(bf)
    wp = np.ascontiguousarray(np.asarray(w_proj, np.float32)).astype(bf)

    # EBm1 = exp(pos_bias) - 1, transposed to [j, i], upper (j > i) zeroed.
    pb = np.asarray(pos_bias, np.float32)
    ebm1 = (np.exp(pb) - 1.0).T.copy()
    jj = np.arange(T)[:, None]
    ii = np.arange(T)[None, :]
    ebm1[jj > ii] = 0.0
    pb8 = ebm1.astype(f8)

    consts = np.zeros((P, 3, P), np.float32)
    consts[:, 0, :] = (np.arange(P)[:, None] <= np.arange(P)[None, :])  # ltri
    consts[:, 1, :] = np.eye(P)
    consts[:, 2, :] = 1.0
    consts = consts.astype(bf)

    shared = dict(wq=wq, wk=wk, wv=wv, wp=wp, pb8=pb8, consts=consts)
    return [dict(xTb=xTb_all[i], **shared) for i in range(B)]


_NC_CACHE = {}


def get_nc():
    if "nc" not in _NC_CACHE:
        _NC_CACHE["nc"] = build_nc()
    return _NC_CACHE["nc"]


def kernel(x, w_attn, w_proj, pos_bias):
    nc = get_nc()
    in_maps = make_in_maps(x, w_attn, w_proj, pos_bias)
    res = run_bass_kernel_spmd(nc, in_maps, core_ids=list(range(B)))
    return np.stack([res.results[i]["out"] for i in range(B)]).astype(np.float32)
